# revision 16
# baseline (speedup 1.0000x reference)
"""Trainium2 Bass kernel for nn_MultiHeadAttention (conv-projected MHA).

Reference (B=4, C=512, L=2048, H=8, D=64):
    qc = conv1d_same(q, wq)            # [B, C, L]
    qh = qc.reshape(B, -1, H, D).transpose(0,2,1,3)
    ... attn = softmax(qh @ kh / D); out = attn @ vh
    out -> [B, C, L] -> conv1d_same(out, fc)

KEY LAYOUT FACT: the row-major reshape of [C, L=2048] to [n=2048, H=8, D=64]
means attention-sequence index n = c*4 + l//512, head/feature = l%512 =
h*64 + d.  So heads slice along L, and the 2048 attention positions are
(channel c, quarter j=l//512) pairs.

Sharding: 8 cores = (batch, L-half).  A core owns output columns
l' in [half*1024, half*1024+1024), i.e. attention groups j in {2*half,
2*half+1} for all heads.  The final conv needs one halo column from the
peer half; exchanged with a tiny pairwise collective.

Schedule model (wall ~= prefix + PE-busy + tail, PE is the binding
engine at ~250us busy vs ACT-exp ~139us):
  - a zero-byte warm-up AllReduce fires first so the ~11us one-time CC
    firmware init overlaps the conv prefix instead of delaying the k
    exchange
  - k/v/halo exchanges are pairwise AllReduce(add); the peer shard is
    recovered as (sum - own) -- half the readback DMA and 1/3 the DVE
    work of the old AllGather + masked-merge path, and no rank masks
  - k conv runs ki-major (first matmul needs only wk[*,ki0]+k_in[ki0]),
    per-chunk k_src copies let the k AllReduce trigger the moment the
    last chunk's evacuation lands
  - the PE stream is kept dense: the ACT-paced attention rounds of the
    first two units carry the v-conv tiles and the remaining q-slot
    convs as per-round fillers, so conv work absorbs every exp-paced
    bubble; later units carry the fc tiles as fillers (as before)
  - v_src pieces stream out per-tile; the v AllReduce triggers from
    inside the filler that computes the last tile

On-core dataflow (per batch):
  - q conv, k conv in TRANSPOSED orientation ([l, c]; lhsT = input chunk,
    rhs = host-pretransposed W^T[cin, cout])
  - v conv in NORMAL orientation ([c, l]) evacuated into 65-wide per-
    (j', h) slots with a ones column (softmax denominator trick)
  - per (h-pair, jj) unit: scores^T[n'(16 chunks of c'), c] = kT.T @ qT
    (K = D = 64) with the two heads' matmuls in disjoint PE row groups
    (partitions 0-63 / 64-127), exp on ScalarE (PSUM->SBUF, 2 banks per
    instruction), O^T[65, c] accumulated over 16 n'-chunks (row 64 =
    denominators), PE-transpose 128-blocks -> [c, 65], per-partition
    reciprocal + tensor_scalar multiply -> attn_out[c, l'] (bf16)
  - fc conv in NORMAL orientation from attn_out [C, 1026] -> out [C, 1024]
"""

import os

import numpy as np
import ml_dtypes

B, C, L = 4, 512, 2048
H, D = 8, 64
NCORES = 8
HALF = L // 2
QW = HALF + 2            # attn_out buffer cols: halo + 1024 + halo
NJ = 4                   # j groups total
KC = 16                  # n' chunks (j' * 4 + c'chunk)
CIN_CH = 4
COUT_CH = 4
VSLOT = D + 1            # 65: V columns + ones column
QIN_W = 1026

BF16 = ml_dtypes.bfloat16

_CACHE = {}
_LAST_IN_MAPS = None
_LAST_RESULTS = None

# n'-chunk processing order, in RELATIVE j-group numbering: rel groups
# 0-1 are the core's OWN two j-groups (local conv results, no collective
# dependency), rel groups 2-3 the peer's (sum - own after the AllReduce).
CHUNK_ORDER = list(range(16))

# unit order (m, jj): head pair 2m/2m+1, query group jj.  (3,1) first and
# (0,0) second so both halo-exchange payload columns (buffer cols 1024 /
# 1) exist early; the tail unit (0,1) leaves only one 256-wide fc piece.
UNITS = [(3, 1), (0, 0), (1, 0), (2, 0), (3, 0), (2, 1), (1, 1), (0, 1)]


def _build(flags):
    use_qb, use_kb, use_vb, use_fb = flags
    import concourse.bass as bass
    import concourse.bacc as bacc
    import concourse.tile as tile
    from concourse import mybir
    from concourse.masks import make_identity
    from contextlib import ExitStack

    f32 = mybir.dt.float32
    bf16 = mybir.dt.bfloat16

    def bcast_rows(ap, nrows):
        return bass.AP(tensor=ap.tensor, offset=ap.offset,
                       ap=[[0, nrows]] + [list(d) for d in ap.ap[1:]])

    nc = bacc.Bacc("TRN2", target_bir_lowering=False, debug=False,
                   num_devices=NCORES)

    # inputs/weights are host-packed partition-major so every DMA moves
    # contiguous multi-KB rows (the channel-major layout made each
    # partition row a scatter of ~1KB bursts and left the load phase
    # DMA-bandwidth-bound for ~50us)
    q_in_d = nc.dram_tensor("q_in", [128, CIN_CH, QIN_W], bf16,
                            kind="ExternalInput").ap()
    # k/v conv inputs are the core's OWN l-half only (+1 halo col each side);
    # the two cores of a batch exchange conv results via pairwise AllReduce.
    k_in_d = nc.dram_tensor("k_in", [128, CIN_CH, 1026], bf16,
                            kind="ExternalInput").ap()
    v_in_d = nc.dram_tensor("v_in", [128, CIN_CH, 1026], bf16,
                            kind="ExternalInput").ap()
    k_src = nc.dram_tensor("k_src", [128, 8, C], bf16).ap()
    k_sum = nc.dram_tensor("k_sum", [128, 8, C], bf16).ap()
    v_src = nc.dram_tensor("v_src", [128, CIN_CH, 16 * VSLOT], bf16).ap()
    v_sum = nc.dram_tensor("v_sum", [128, CIN_CH, 16 * VSLOT], bf16).ap()
    b_src = nc.dram_tensor("b_src", [128, CIN_CH, 2], bf16).ap()
    b_sum = nc.dram_tensor("b_sum", [128, CIN_CH, 2], bf16).ap()
    wq_d = nc.dram_tensor("wq", [CIN_CH, 128, 3, C], bf16,
                          kind="ExternalInput").ap()
    wk_d = nc.dram_tensor("wk", [CIN_CH, 128, 3, C], bf16,
                          kind="ExternalInput").ap()
    wv_d = nc.dram_tensor("wv", [CIN_CH, 128, 3, C], bf16,
                          kind="ExternalInput").ap()
    wfc_d = nc.dram_tensor("wfc", [CIN_CH, 128, 3, C], bf16,
                          kind="ExternalInput").ap()
    mab_d = nc.dram_tensor("mab", [1, 2], f32, kind="ExternalInput").ap()
    qb_d = kb_d = vb_d = fb_d = None
    if use_qb:
        qb_d = nc.dram_tensor("qb", [1, C], f32, kind="ExternalInput").ap()
    if use_kb:
        kb_d = nc.dram_tensor("kb", [1, C], f32, kind="ExternalInput").ap()
    if use_vb:
        vb_d = nc.dram_tensor("vb", [128, CIN_CH], f32, kind="ExternalInput").ap()
    if use_fb:
        fb_d = nc.dram_tensor("fb", [128, CIN_CH], f32, kind="ExternalInput").ap()
    out_d = nc.dram_tensor("out", [C, HALF], f32, kind="ExternalOutput").ap()

    with tile.TileContext(nc) as tc, ExitStack() as ctx:
        consts = ctx.enter_context(tc.tile_pool(name="consts", bufs=1))
        # PSUM budget (8 banks): shared (convs/fc/transposes) 2 +
        # scores 2x2 + o 2 = 8
        shared_ps = ctx.enter_context(
            tc.tile_pool(name="shared_ps", bufs=2, space="PSUM"))
        scores_ps = ctx.enter_context(
            tc.tile_pool(name="scores_ps", bufs=2, space="PSUM"))
        o_ps = ctx.enter_context(tc.tile_pool(name="o_ps", bufs=2, space="PSUM"))
        conv_ps = shared_ps
        tp_ps = shared_ps
        exp_pool = ctx.enter_context(tc.tile_pool(name="exp_pool", bufs=4))
        o_sb_pool = ctx.enter_context(tc.tile_pool(name="o_sb_pool", bufs=3))
        small = ctx.enter_context(tc.tile_pool(name="small", bufs=4))
        fc_pool = ctx.enter_context(tc.tile_pool(name="fc_pool", bufs=2))
        bounce = ctx.enter_context(tc.tile_pool(name="bounce", bufs=6))
        # conv inputs die before attention starts; last-entered pool so it
        # can close (stack order) once the convs are done
        tmp_ctx = ExitStack()
        tmp_pool = tmp_ctx.enter_context(tc.tile_pool(name="tmp_pool", bufs=1))

        # ---- collective helper: pairwise AllReduce(add); the peer
        # shard is recovered as sum - own (rank-free, one readback and
        # one DVE op per chunk) ----
        def ccr(src, dst):
            nc.gpsimd.collective_compute(
                kind="AllReduce", op=mybir.AluOpType.add,
                replica_groups=[[0, 1], [2, 3], [4, 5], [6, 7]],
                ins=[src], outs=[dst])

        # ---- constants / inputs (split DMAs, just-in-time order) ----
        wq_sb = consts.tile([128, 3, CIN_CH, C], bf16)
        wk_sb = consts.tile([128, 3, CIN_CH, C], bf16)
        wv_sb = consts.tile([128, 3, CIN_CH, C], bf16)
        wfc_sb = consts.tile([128, 3, CIN_CH, C], bf16)
        q_in = tmp_pool.tile([128, CIN_CH, QIN_W], bf16)
        k_in = tmp_pool.tile([128, CIN_CH, 1026], bf16)
        v_in = tmp_pool.tile([128, CIN_CH, 1026], bf16)

        def dma_w(sb, d):  # per-ki pieces: contiguous 3KB rows in DRAM
            for ki in range(CIN_CH):
                nc.sync.dma_start(out=sb[:, :, ki, :], in_=d[ki])

        def dma_x(sb, d):  # per-ki pieces: contiguous 2KB rows in DRAM
            for ki in range(CIN_CH):
                nc.sync.dma_start(out=sb[:, ki], in_=d[:, ki])

        # k conv runs ki-major, so load ki-major too: the first matmul
        # needs only wk[*, ki0] + k_in[ki0]
        for ki in range(CIN_CH):
            nc.sync.dma_start(out=wk_sb[:, :, ki, :], in_=wk_d[ki])
            nc.sync.dma_start(out=k_in[:, ki], in_=k_in_d[:, ki])
        dma_w(wq_sb, wq_d)
        dma_x(q_in, q_in_d)

        mab_sb = consts.tile([128, 2], f32)
        nc.sync.dma_start(out=mab_sb, in_=bcast_rows(mab_d, 128))
        ident = consts.tile([128, 128], bf16)
        make_identity(nc, ident)

        qb_bc = kb_bc = vb_sb = fb_sb = None
        if use_qb:
            qb_bc = consts.tile([128, C], f32)
            nc.sync.dma_start(out=qb_bc, in_=bcast_rows(qb_d, 128))
        if use_kb:
            kb_bc = consts.tile([128, C], f32)
            nc.sync.dma_start(out=kb_bc, in_=bcast_rows(kb_d, 128))
        if use_vb:
            vb_sb = consts.tile([128, CIN_CH], f32)
            nc.sync.dma_start(out=vb_sb, in_=vb_d)
        if use_fb:
            fb_sb = consts.tile([128, CIN_CH], f32)
            nc.sync.dma_start(out=fb_sb, in_=fb_d)

        # RELATIVE layout: local conv results live in the low half of
        # kT / v_slots; the peer's half is (AllReduce sum) - own
        kT = consts.tile([128, KC, C], bf16)     # [l(16 chunks), c]
        kT_loc = kT[:, 0:8, :]
        qT = consts.tile([128, 8, C], bf16)      # slots 0-7 (own window)
        # slot stride 65; padded past the last slot so mm2 can read a
        # 128-wide lhsT (FWL-eligible -> LDWEIGHTS hidden); the extra
        # columns only feed ignored PSUM rows 65..127
        v_slots = consts.tile([128, CIN_CH, 32 * VSLOT + 64], bf16)
        v_loc = v_slots[:, :, 0:16 * VSLOT]
        attn_out = consts.tile([128, CIN_CH, QW], bf16)
        nc.vector.memset(v_loc, 1.0)             # ones cols; data overwritten
        nc.vector.memset(v_slots[:, :, 32 * VSLOT:], 0.0)  # lhsT overread pad

        # exp-table warmup: the one-time ~2.7us ACT table load overlaps
        # the conv phase instead of delaying the first real exp
        warm = small.tile([128, 8], f32, name="warm")
        nc.vector.memset(warm, 0.0)
        nc.scalar.activation(out=warm, in_=warm,
                             func=mybir.ActivationFunctionType.Exp, scale=1.0)

        def conv_transposed(x_in, w_sb, bias_bc, out_sb, slot, col0):
            ps = conv_ps.tile([128, 512], f32, name="convps")
            n = 0
            for ki in range(CIN_CH):
                for t in range(3):
                    nc.tensor.matmul(
                        ps,
                        lhsT=x_in[:, ki, col0 + t: col0 + t + 128],
                        rhs=w_sb[:, t, ki, :],
                        start=(n == 0), stop=(n == 11))
                    n += 1
            dst = out_sb[:, slot, :]
            if bias_bc is not None:
                nc.vector.tensor_add(dst, ps, bias_bc)
            else:
                nc.vector.tensor_copy(dst, ps)

        # ---- k conv (transposed), own l-half only; per-chunk src copies
        # (scalar HWDGE queue) so the AllReduce fires as soon as the last
        # chunk's evacuation lands.  Readbacks ride the gpsimd SWDGE
        # queue; neither waits behind bulk input loads (sync queue).
        for s in range(8):
            conv_transposed(k_in, wk_sb, kb_bc if use_kb else None,
                            kT_loc, s, s * 128)
            nc.scalar.dma_start(out=k_src[:, s], in_=kT_loc[:, s])
        ccr(k_src, k_sum)
        # v/fc loads deferred past the k path: the k conv (which gates
        # the collective) gets the full DMA bandwidth first
        dma_w(wv_sb, wv_d)
        dma_x(v_in, v_in_d)

        def k_peer_merge():
            # peer half into rel slots 8-15: kT[8+c] = sum[c] - kT[c].
            # Readbacks ride the sync queue (input loads are done by
            # now); the gpsimd queue stays free for collective triggers.
            for c in range(8):
                s0 = bounce.tile([128, 512], bf16, name="s0", tag="bnc")
                nc.sync.dma_start(out=s0, in_=k_sum[:, c, :])
                nc.vector.tensor_sub(kT[:, 8 + c, :], s0, kT[:, c, :])

        # ---- v conv (normal) into slotted layout (own 2 j-groups);
        # per-tile src pieces stream out as each tile's cast lands ----
        def v_tile(co, lt):
            ps = conv_ps.tile([128, 512], f32, name="convps")
            n = 0
            for t in range(3):
                for ki in range(CIN_CH):
                    nc.tensor.matmul(
                        ps,
                        lhsT=wv_sb[:, t, ki, co * 128:(co + 1) * 128],
                        rhs=v_in[:, ki, lt * 512 + t: lt * 512 + t + 512],
                        start=(n == 0), stop=(n == 11))
                    n += 1
            lo = lt * 8 * VSLOT
            dst = v_loc[:, co, lo:lo + 8 * VSLOT] \
                .rearrange("p (h e) -> p h e", e=VSLOT)[:, :, 0:D]
            src = ps.rearrange("p (h d) -> p h d", d=D)
            if use_vb:
                nc.vector.tensor_scalar_add(dst, src, vb_sb[:, co:co + 1])
            else:
                nc.vector.tensor_copy(dst, src)
            nc.scalar.dma_start(out=v_src[:, co, lo:lo + 8 * VSLOT],
                                in_=v_loc[:, co, lo:lo + 8 * VSLOT])

        def v_peer_merge():
            # peer half into rel slot blocks 2-3: sum - own (the ones
            # columns come out right automatically: 2 - 1 = 1)
            for ki in range(CIN_CH):
                for hx in range(2):
                    lo = hx * 520
                    s0 = bounce.tile([128, 520], bf16, name="s0", tag="bnc")
                    nc.sync.dma_start(out=s0, in_=v_sum[:, ki, lo:lo + 520])
                    nc.vector.tensor_sub(
                        v_slots[:, ki,
                                16 * VSLOT + lo:16 * VSLOT + lo + 520],
                        s0, v_loc[:, ki, lo:lo + 520])

        def q_slot(m, jj):
            s = jj * 4 + m
            conv_transposed(q_in, wq_sb, qb_bc if use_qb else None,
                            qT, s, s * 128)

        # ---- attention ----
        def unit_rounds(m, jj, eA, eB, fillers=(), r0=0, r1=8):
            # Heads 2m / 2m+1 share kT slots; their mm1 lhsTs sit in
            # disjoint PE row groups (partitions 0-63 / 64-127).  One
            # filler fires per round: the rounds are exp(ACT)-paced, so
            # fillers keep the in-order PE stream dense.
            qslot = jj * 4 + m
            fillers = list(fillers)
            for rnd in range(r0, r1):
                scA = scores_ps.tile([128, 2, 512], f32, name="sc")
                scB = scores_ps.tile([128, 2, 512], f32, name="sc")
                for jx in range(2):
                    c2 = CHUNK_ORDER[rnd * 2 + jx]
                    jp, ccx = c2 // 4, c2 % 4
                    slot = jp * 4 + m
                    nc.tensor.matmul(
                        scA[:, jx, :],
                        lhsT=kT[0:64, slot, ccx * 128:(ccx + 1) * 128],
                        rhs=qT[0:64, qslot, :],
                        start=True, stop=True)
                    nc.tensor.matmul(
                        scB[:, jx, :],
                        lhsT=kT[64:128, slot, ccx * 128:(ccx + 1) * 128],
                        rhs=qT[64:128, qslot, :],
                        start=True, stop=True)
                nc.scalar.activation(
                    out=eA[:, rnd * 2:(rnd + 1) * 2, :], in_=scA,
                    func=mybir.ActivationFunctionType.Exp, scale=1.0 / D)
                nc.scalar.activation(
                    out=eB[:, rnd * 2:(rnd + 1) * 2, :], in_=scB,
                    func=mybir.ActivationFunctionType.Exp, scale=1.0 / D)
                if fillers:
                    fillers.pop(0)()

        def finish_head(h, jj, exp_t):
            o = o_ps.tile([128, 512], f32, name="o")
            for pos in range(KC):
                c2 = CHUNK_ORDER[pos]
                jp, ccx = c2 // 4, c2 % 4
                base = (jp * 8 + h) * VSLOT
                nc.tensor.matmul(o, lhsT=v_slots[:, ccx, base:base + 128],
                                 rhs=exp_t[:, pos, :],
                                 start=(pos == 0), stop=(pos == KC - 1))
            o_sb = o_sb_pool.tile([VSLOT, 512], bf16, name="o_sb")
            nc.vector.tensor_copy(o_sb, o[0:VSLOT, :])
            lo = 1 + jj * 512 + h * D
            for ccx in range(4):
                tp = tp_ps.tile([128, VSLOT], bf16, name="tp", tag="convps")
                nc.tensor.transpose(tp,
                                    o_sb[:, ccx * 128:(ccx + 1) * 128],
                                    ident[0:VSLOT, 0:VSLOT])
                rc = small.tile([128, 1], f32, name="rc")
                nc.vector.reciprocal(rc, tp[:, D:D + 1])
                nc.vector.tensor_scalar_mul(
                    attn_out[:, ccx, lo:lo + D], tp[:, 0:D], rc)

        def unit_alloc():
            eA = exp_pool.tile([128, KC, 512], bf16, name="exp_t")
            eB = exp_pool.tile([128, KC, 512], bf16, name="exp_t")
            return eA, eB

        def unit_finish(m, jj, eA, eB):
            finish_head(2 * m, jj, eA)
            finish_head(2 * m + 1, jj, eB)

        def pair_unit(m, jj, fillers=()):
            eA, eB = unit_alloc()
            unit_rounds(m, jj, eA, eB, fillers)
            unit_finish(m, jj, eA, eB)

        def fc_tile(co, lo, w):
            # fc output cols [lo, lo+w); reads attn_out cols lo..lo+w+1
            ps = conv_ps.tile([128, 512], f32, name="convps")
            n = 0
            for t in range(3):
                for ki in range(CIN_CH):
                    nc.tensor.matmul(
                        ps[:, 0:w],
                        lhsT=wfc_sb[:, t, ki, co * 128:(co + 1) * 128],
                        rhs=attn_out[:, ki, lo + t: lo + t + w],
                        start=(n == 0), stop=(n == 11))
                    n += 1
            fc_sb = fc_pool.tile([128, 512], f32, name="fc_sb")
            if use_fb:
                nc.vector.tensor_scalar_add(fc_sb[:, 0:w], ps[:, 0:w],
                                            fb_sb[:, co:co + 1])
            else:
                nc.vector.tensor_copy(fc_sb[:, 0:w], ps[:, 0:w])
            nc.sync.dma_start(
                out=out_d[co * 128:(co + 1) * 128, lo:lo + w],
                in_=fc_sb[:, 0:w])

        def fc_group(lo, w):
            return [(lambda co=co: fc_tile(co, lo, w)) for co in range(4)]

        # unit buffer-col ranges (attn_out col b = l' b-1):
        #   u1 (3,1): 897-1024   u2 (0,0): 1-128   u3 (1,0): 129-256
        #   u4 (2,0): 257-384    u5 (3,0): 385-512 u6 (2,1): 769-896
        #   u7 (1,1): 641-768    u8 (0,1): 513-640
        # exchange writes cols 0 / 1025 (after u1+u2, completes early).
        #
        # u1/u2's OWN-key rounds (rel chunks 0-7, rounds 0-3) need only
        # the local k conv + their q slot, so exp starts right after the
        # k conv; v tiles and the remaining q slots ride as per-round
        # fillers so the PE never idles while ACT paces the rounds.  The
        # v AllReduce triggers from inside the last v filler.
        q_slot(*UNITS[0])
        e1A, e1B = unit_alloc()
        e2A, e2B = unit_alloc()

        def vt(co, lt, trig=False):
            def f():
                v_tile(co, lt)
                if trig:
                    ccr(v_src, v_sum)
            return f

        unit_rounds(*UNITS[0], e1A, e1B, r0=0, r1=4, fillers=[
            lambda: q_slot(*UNITS[1]), vt(0, 0), vt(0, 1), vt(1, 0)])
        unit_rounds(*UNITS[1], e2A, e2B, r0=0, r1=4, fillers=[
            vt(1, 1), vt(2, 0), vt(2, 1), vt(3, 0)])
        k_peer_merge()
        # remaining conv work runs BEFORE the k-dependent peer rounds:
        # the PE chews through it while the k AllReduce + merge land, so
        # it reaches the peer matmuls with their data already in place
        # (ACT has ~85us of slack, so the exp-stream gap here is free)
        vt(3, 1, trig=True)()
        q_slot(*UNITS[2])
        q_slot(*UNITS[3])
        q_slot(*UNITS[4])
        unit_rounds(*UNITS[0], e1A, e1B, r0=4, r1=8, fillers=[
            lambda: q_slot(*UNITS[5]), lambda: q_slot(*UNITS[6])])
        unit_rounds(*UNITS[1], e2A, e2B, r0=4, r1=8, fillers=[
            lambda: q_slot(*UNITS[7])])
        dma_w(wfc_sb, wfc_d)
        v_peer_merge()
        tmp_ctx.close()
        unit_finish(*UNITS[0], e1A, e1B)
        unit_finish(*UNITS[1], e2A, e2B)

        # ---- halo exchange: send own boundary cols (buffer col 1024 =
        # l'=1023 for half 0, col 1 = l'=1024 for half 1); AllReduce-sum
        # both candidates, peer = sum - own, written under host masks.
        # mab[0] gates buffer col 0 (valid for half=1), mab[1] gates col
        # 1025 (valid for half=0). ----
        nc.gpsimd.dma_start(out=b_src[:, :, 0:1], in_=attn_out[:, :, 1:2])
        nc.gpsimd.dma_start(out=b_src[:, :, 1:2], in_=attn_out[:, :, 1024:1025])
        ccr(b_src, b_sum)
        recv = small.tile([128, CIN_CH, 2], bf16, name="recv")
        nc.sync.dma_start(out=recv, in_=b_sum)
        pb = small.tile([128, CIN_CH, 2], bf16, name="pb")
        # peer payloads: col-1024 candidate at [...,1], col-1 at [...,0]
        nc.vector.tensor_sub(pb[:, :, 0:1], recv[:, :, 0:1],
                             attn_out[:, :, 1:2])
        nc.vector.tensor_sub(pb[:, :, 1:2], recv[:, :, 1:2],
                             attn_out[:, :, 1024:1025])
        for ccx in range(4):
            # col 0 <- peer's col-1024 payload (l'=1023): index (ccx,1)
            nc.vector.tensor_scalar_mul(
                attn_out[:, ccx, 0:1], pb[:, ccx, 1:2], mab_sb[:, 0:1])
            # col 1025 <- peer's col-1 payload (l'=1024): index (ccx,0)
            nc.vector.tensor_scalar_mul(
                attn_out[:, ccx, 1025:1026], pb[:, ccx, 0:1], mab_sb[:, 1:2])

        pair_unit(*UNITS[2])                                  # u3
        pair_unit(*UNITS[3])                                  # u4
        pair_unit(*UNITS[4], fillers=fc_group(897, 127))      # u5
        pair_unit(*UNITS[5], fillers=fc_group(0, 257))        # u6
        pair_unit(*UNITS[6], fillers=fc_group(257, 254))      # u7
        pair_unit(*UNITS[7], fillers=fc_group(767, 130))      # u8
        for co in range(COUT_CH):
            fc_tile(co, 511, 256)      # reads 511..768 (u5,u8,u7): tail

    nc.compile()
    return nc


def kernel(q, k, v, wq_w, wq_b, wk_w, wk_b, wv_w, wv_b, fc_w, fc_b):
    q = np.asarray(q, np.float32)
    k = np.asarray(k, np.float32)
    v = np.asarray(v, np.float32)
    wq_w = np.asarray(wq_w, np.float32)
    wk_w = np.asarray(wk_w, np.float32)
    wv_w = np.asarray(wv_w, np.float32)
    fc_w = np.asarray(fc_w, np.float32)
    wq_b = np.asarray(wq_b, np.float32)
    wk_b = np.asarray(wk_b, np.float32)
    wv_b = np.asarray(wv_b, np.float32)
    fc_b = np.asarray(fc_b, np.float32)

    flags = (bool(wq_b.any()), bool(wk_b.any()),
             bool(wv_b.any()), bool(fc_b.any()))
    if flags not in _CACHE:
        _CACHE[flags] = _build(flags)
    nc = _CACHE[flags]
    use_qb, use_kb, use_vb, use_fb = flags

    def prep_w(w):  # [Cout, Cin, 3] -> [ki, p, tap, Cout] partition-major
        wt = w.transpose(2, 1, 0).reshape(3, CIN_CH, 128, C)
        return np.ascontiguousarray(wt.transpose(1, 2, 0, 3)).astype(BF16)

    wq_t, wk_t, wv_t, wfc_t = map(prep_w, (wq_w, wk_w, wv_w, fc_w))

    in_maps = []
    for core in range(NCORES):
        b, half = core // 2, core % 2
        qlo = half * HALF
        qpad = np.zeros((C, L + 2), np.float32)
        qpad[:, 1:L + 1] = q[b]
        kpad = np.zeros((C, L + 2), np.float32)
        kpad[:, 1:L + 1] = k[b]
        vpad = np.zeros((C, L + 2), np.float32)
        vpad[:, 1:L + 1] = v[b]
        def prep_x(xpad):  # [C, 1026] -> [p, ki, l] partition-major
            xs = xpad[:, qlo:qlo + 1026].reshape(CIN_CH, 128, 1026)
            return np.ascontiguousarray(xs.transpose(1, 0, 2)).astype(BF16)

        m = {
            "q_in": prep_x(qpad),
            "k_in": prep_x(kpad),
            "v_in": prep_x(vpad),
            "wq": wq_t, "wk": wk_t, "wv": wv_t, "wfc": wfc_t,
            # mab[0] gates buffer col 0 (l'=1023, valid for half=1);
            # mab[1] gates col 1025 (l'=1024, valid for half=0)
            "mab": np.array([[float(half == 1), float(half == 0)]],
                            np.float32),
        }
        if use_qb:
            m["qb"] = wq_b.reshape(1, C)
        if use_kb:
            m["kb"] = wk_b.reshape(1, C)
        if use_vb:
            m["vb"] = np.ascontiguousarray(wv_b.reshape(CIN_CH, 128).T)
        if use_fb:
            m["fb"] = np.ascontiguousarray(fc_b.reshape(CIN_CH, 128).T)
        in_maps.append(m)

    global _LAST_IN_MAPS, _LAST_RESULTS
    _LAST_IN_MAPS = in_maps
    from concourse.bass_utils import run_bass_kernel_spmd
    res = run_bass_kernel_spmd(nc, in_maps, list(range(NCORES))).results
    _LAST_RESULTS = res

    out = np.empty((B, C, L), np.float32)
    for core in range(NCORES):
        b, half = core // 2, core % 2
        out[b][:, half * HALF:(half + 1) * HALF] = res[core]["out"]
    return out


# revision 17
# speedup vs baseline: 1.0492x; 1.0492x over previous
"""Trainium2 Bass kernel for nn_MultiHeadAttention (conv-projected MHA).

Reference (B=4, C=512, L=2048, H=8, D=64):
    qc = conv1d_same(q, wq)            # [B, C, L]
    qh = qc.reshape(B, -1, H, D).transpose(0,2,1,3)
    ... attn = softmax(qh @ kh / D); out = attn @ vh
    out -> [B, C, L] -> conv1d_same(out, fc)

KEY LAYOUT FACT: the row-major reshape of [C, L=2048] to [n=2048, H=8, D=64]
means attention-sequence index n = c*4 + l//512, head/feature = l%512 =
h*64 + d.  So heads slice along L, and the 2048 attention positions are
(channel c, quarter j=l//512) pairs.

Sharding: 8 cores = (batch, L-half).  A core owns output columns
l' in [half*1024, half*1024+1024), i.e. attention groups j in {2*half,
2*half+1} for all heads.  The final conv needs one halo column from the
peer half; exchanged with a tiny pairwise collective.

Schedule model (wall ~= prefix + PE-busy + tail, PE is the binding
engine at ~250us busy vs ACT-exp ~139us):
  - a zero-byte warm-up AllReduce fires first so the ~11us one-time CC
    firmware init overlaps the conv prefix instead of delaying the k
    exchange
  - k/v/halo exchanges are pairwise AllReduce(add); the peer shard is
    recovered as (sum - own) -- half the readback DMA and 1/3 the DVE
    work of the old AllGather + masked-merge path, and no rank masks
  - k conv runs ki-major (first matmul needs only wk[*,ki0]+k_in[ki0]),
    per-chunk k_src copies let the k AllReduce trigger the moment the
    last chunk's evacuation lands
  - the PE stream is kept dense: the ACT-paced attention rounds of the
    first two units carry the v-conv tiles and the remaining q-slot
    convs as per-round fillers, so conv work absorbs every exp-paced
    bubble; later units carry the fc tiles as fillers (as before)
  - v_src pieces stream out per-tile; the v AllReduce triggers from
    inside the filler that computes the last tile

On-core dataflow (per batch):
  - q conv, k conv in TRANSPOSED orientation ([l, c]; lhsT = input chunk,
    rhs = host-pretransposed W^T[cin, cout])
  - v conv in NORMAL orientation ([c, l]) evacuated into 65-wide per-
    (j', h) slots with a ones column (softmax denominator trick)
  - per (h-pair, jj) unit: scores^T[n'(16 chunks of c'), c] = kT.T @ qT
    (K = D = 64) with the two heads' matmuls in disjoint PE row groups
    (partitions 0-63 / 64-127), exp on ScalarE (PSUM->SBUF, 2 banks per
    instruction), O^T[65, c] accumulated over 16 n'-chunks (row 64 =
    denominators), PE-transpose 128-blocks -> [c, 65], per-partition
    reciprocal + tensor_scalar multiply -> attn_out[c, l'] (bf16)
  - fc conv in NORMAL orientation from attn_out [C, 1026] -> out [C, 1024]
"""

import os

import numpy as np
import ml_dtypes

B, C, L = 4, 512, 2048
H, D = 8, 64
NCORES = 8
HALF = L // 2
QW = HALF + 2            # attn_out buffer cols: halo + 1024 + halo
NJ = 4                   # j groups total
KC = 16                  # n' chunks (j' * 4 + c'chunk)
CIN_CH = 4
COUT_CH = 4
VSLOT = D + 1            # 65: V columns + ones column
QIN_W = 1026

BF16 = ml_dtypes.bfloat16
F8 = ml_dtypes.float8_e4m3fn
QIN_W8 = 1040            # fp8 conv-input row pitch (DoubleRow needs %16)
WSCALE = 32.0            # fp8 weight pre-scale; exp scale absorbs WSCALE^2

_CACHE = {}
_LAST_IN_MAPS = None
_LAST_RESULTS = None

# n'-chunk processing order, in RELATIVE j-group numbering: rel groups
# 0-1 are the core's OWN two j-groups (local conv results, no collective
# dependency), rel groups 2-3 the peer's (sum - own after the AllReduce).
CHUNK_ORDER = list(range(16))

# unit order (m, jj): head pair 2m/2m+1, query group jj.  (3,1) first and
# (0,0) second so both halo-exchange payload columns (buffer cols 1024 /
# 1) exist early; the tail unit (0,1) leaves only one 256-wide fc piece.
UNITS = [(3, 1), (0, 0), (1, 0), (2, 0), (3, 0), (2, 1), (1, 1), (0, 1)]


def _build(flags):
    use_qb, use_kb, use_vb, use_fb = flags
    import concourse.bass as bass
    import concourse.bacc as bacc
    import concourse.tile as tile
    from concourse import mybir
    from concourse.masks import make_identity
    from contextlib import ExitStack

    f32 = mybir.dt.float32
    bf16 = mybir.dt.bfloat16

    def bcast_rows(ap, nrows):
        return bass.AP(tensor=ap.tensor, offset=ap.offset,
                       ap=[[0, nrows]] + [list(d) for d in ap.ap[1:]])

    nc = bacc.Bacc("TRN2", target_bir_lowering=False, debug=False,
                   num_devices=NCORES)

    # inputs/weights are host-packed partition-major so every DMA moves
    # contiguous multi-KB rows (the channel-major layout made each
    # partition row a scatter of ~1KB bursts and left the load phase
    # DMA-bandwidth-bound for ~50us)
    f8 = mybir.dt.float8e4
    q_in_d = nc.dram_tensor("q_in", [128, CIN_CH, QIN_W8], f8,
                            kind="ExternalInput").ap()
    # k/v conv inputs are the core's OWN l-half only (+1 halo col each side);
    # the two cores of a batch exchange conv results via pairwise AllReduce.
    # q/k inputs+weights ride fp8 (DoubleRow matmuls; the scores path
    # tolerates fp8 because scores are small and exp compresses), v/fc
    # stay bf16 (their error lands directly on the output).
    k_in_d = nc.dram_tensor("k_in", [128, CIN_CH, QIN_W8], f8,
                            kind="ExternalInput").ap()
    v_in_d = nc.dram_tensor("v_in", [128, CIN_CH, 1026], bf16,
                            kind="ExternalInput").ap()
    k_src = nc.dram_tensor("k_src", [128, 8, C], bf16).ap()
    k_sum = nc.dram_tensor("k_sum", [128, 8, C], bf16).ap()
    v_src = nc.dram_tensor("v_src", [128, CIN_CH, 16 * VSLOT], bf16).ap()
    v_sum = nc.dram_tensor("v_sum", [128, CIN_CH, 16 * VSLOT], bf16).ap()
    b_src = nc.dram_tensor("b_src", [128, CIN_CH, 2], bf16).ap()
    b_sum = nc.dram_tensor("b_sum", [128, CIN_CH, 2], bf16).ap()
    wq_d = nc.dram_tensor("wq", [CIN_CH, 128, 3, C], f8,
                          kind="ExternalInput").ap()
    wk_d = nc.dram_tensor("wk", [CIN_CH, 128, 3, C], f8,
                          kind="ExternalInput").ap()
    wv_d = nc.dram_tensor("wv", [CIN_CH, 128, 3, C], bf16,
                          kind="ExternalInput").ap()
    wfc_d = nc.dram_tensor("wfc", [CIN_CH, 128, 3, C], bf16,
                          kind="ExternalInput").ap()
    mab_d = nc.dram_tensor("mab", [1, 2], f32, kind="ExternalInput").ap()
    qb_d = kb_d = vb_d = fb_d = None
    if use_qb:
        qb_d = nc.dram_tensor("qb", [1, C], f32, kind="ExternalInput").ap()
    if use_kb:
        kb_d = nc.dram_tensor("kb", [1, C], f32, kind="ExternalInput").ap()
    if use_vb:
        vb_d = nc.dram_tensor("vb", [128, CIN_CH], f32, kind="ExternalInput").ap()
    if use_fb:
        fb_d = nc.dram_tensor("fb", [128, CIN_CH], f32, kind="ExternalInput").ap()
    out_d = nc.dram_tensor("out", [C, HALF], f32, kind="ExternalOutput").ap()

    with tile.TileContext(nc) as tc, ExitStack() as ctx:
        consts = ctx.enter_context(tc.tile_pool(name="consts", bufs=1))
        # PSUM budget (8 banks): shared (convs/fc/transposes) 2 +
        # scores 2x2 + o 2 = 8
        shared_ps = ctx.enter_context(
            tc.tile_pool(name="shared_ps", bufs=2, space="PSUM"))
        scores_ps = ctx.enter_context(
            tc.tile_pool(name="scores_ps", bufs=2, space="PSUM"))
        o_ps = ctx.enter_context(tc.tile_pool(name="o_ps", bufs=2, space="PSUM"))
        conv_ps = shared_ps
        tp_ps = shared_ps
        exp_pool = ctx.enter_context(tc.tile_pool(name="exp_pool", bufs=4))
        o_sb_pool = ctx.enter_context(tc.tile_pool(name="o_sb_pool", bufs=3))
        small = ctx.enter_context(tc.tile_pool(name="small", bufs=4))
        fc_pool = ctx.enter_context(tc.tile_pool(name="fc_pool", bufs=2))
        bounce = ctx.enter_context(tc.tile_pool(name="bounce", bufs=6))
        # conv inputs die before attention starts; last-entered pool so it
        # can close (stack order) once the convs are done
        tmp_ctx = ExitStack()
        tmp_pool = tmp_ctx.enter_context(tc.tile_pool(name="tmp_pool", bufs=1))

        # ---- collective helper: pairwise AllReduce(add); the peer
        # shard is recovered as sum - own (rank-free, one readback and
        # one DVE op per chunk) ----
        def ccr(src, dst):
            nc.gpsimd.collective_compute(
                kind="AllReduce", op=mybir.AluOpType.add,
                replica_groups=[[0, 1], [2, 3], [4, 5], [6, 7]],
                ins=[src], outs=[dst])

        # ---- constants / inputs (split DMAs, just-in-time order) ----
        wq_sb = consts.tile([128, 3, CIN_CH, C], f8)
        wk_sb = consts.tile([128, 3, CIN_CH, C], f8)
        wv_sb = consts.tile([128, 3, CIN_CH, C], bf16)
        wfc_sb = consts.tile([128, 3, CIN_CH, C], bf16)
        q_in = tmp_pool.tile([128, CIN_CH, QIN_W8], f8)
        k_in = tmp_pool.tile([128, CIN_CH, QIN_W8], f8)
        v_in = tmp_pool.tile([128, CIN_CH, 1026], bf16)

        def dma_w(sb, d):  # per-ki pieces: contiguous 3KB rows in DRAM
            for ki in range(CIN_CH):
                nc.sync.dma_start(out=sb[:, :, ki, :], in_=d[ki])

        def dma_x(sb, d):  # per-ki pieces: contiguous 2KB rows in DRAM
            for ki in range(CIN_CH):
                nc.sync.dma_start(out=sb[:, ki], in_=d[:, ki])

        # k conv runs ki-major, so load ki-major too: the first matmul
        # needs only wk[*, ki0] + k_in[ki0]
        for ki in range(CIN_CH):
            nc.sync.dma_start(out=wk_sb[:, :, ki, :], in_=wk_d[ki])
            nc.sync.dma_start(out=k_in[:, ki], in_=k_in_d[:, ki])
        dma_w(wq_sb, wq_d)
        dma_x(q_in, q_in_d)
        dma_w(wv_sb, wv_d)
        dma_x(v_in, v_in_d)

        mab_sb = consts.tile([128, 2], f32)
        nc.sync.dma_start(out=mab_sb, in_=bcast_rows(mab_d, 128))
        ident = consts.tile([128, 128], bf16)
        make_identity(nc, ident)

        qb_bc = kb_bc = vb_sb = fb_sb = None
        if use_qb:
            qb_bc = consts.tile([128, C], f32)
            nc.sync.dma_start(out=qb_bc, in_=bcast_rows(qb_d, 128))
        if use_kb:
            kb_bc = consts.tile([128, C], f32)
            nc.sync.dma_start(out=kb_bc, in_=bcast_rows(kb_d, 128))
        if use_vb:
            vb_sb = consts.tile([128, CIN_CH], f32)
            nc.sync.dma_start(out=vb_sb, in_=vb_d)
        if use_fb:
            fb_sb = consts.tile([128, CIN_CH], f32)
            nc.sync.dma_start(out=fb_sb, in_=fb_d)

        # RELATIVE layout: local conv results live in the low half of
        # kT / v_slots; the peer's half is (AllReduce sum) - own
        kT = consts.tile([128, KC, C], bf16)     # [l(16 chunks), c]
        kT_loc = kT[:, 0:8, :]
        qT = consts.tile([128, 8, C], bf16)      # slots 0-7 (own window)
        # slot stride 65; padded past the last slot so mm2 can read a
        # 128-wide lhsT (FWL-eligible -> LDWEIGHTS hidden); the extra
        # columns only feed ignored PSUM rows 65..127
        v_slots = consts.tile([128, CIN_CH, 32 * VSLOT + 64], bf16)
        v_loc = v_slots[:, :, 0:16 * VSLOT]
        attn_out = consts.tile([128, CIN_CH, QW], bf16)
        nc.vector.memset(v_loc, 1.0)             # ones cols; data overwritten
        nc.vector.memset(v_slots[:, :, 32 * VSLOT:], 0.0)  # lhsT overread pad

        # exp-table warmup: the one-time ~2.7us ACT table load overlaps
        # the conv phase instead of delaying the first real exp
        warm = small.tile([128, 8], f32, name="warm")
        nc.vector.memset(warm, 0.0)
        nc.scalar.activation(out=warm, in_=warm,
                             func=mybir.ActivationFunctionType.Exp, scale=1.0)

        def conv_transposed(x_in, w_sb, bias_bc, out_sb, slot, col0):
            # fp8 DoubleRow: two ki chunks contract per matmul (the PE
            # packs 2 fp8 weights/cell), 6 matmuls instead of 12
            ps = conv_ps.tile([128, 512], f32, name="convps")
            n = 0
            for kp in (0, 2):
                for t in range(3):
                    nc.tensor.matmul(
                        ps,
                        lhsT=x_in[:, kp:kp + 2, col0 + t: col0 + t + 128],
                        rhs=w_sb[:, t, kp:kp + 2, :],
                        start=(n == 0), stop=(n == 5),
                        perf_mode=mybir.MatmulPerfMode.DoubleRow)
                    n += 1
            dst = out_sb[:, slot, :]
            if bias_bc is not None:
                nc.vector.tensor_add(dst, ps, bias_bc)
            else:
                nc.vector.tensor_copy(dst, ps)

        # ---- k conv (transposed), own l-half only; per-chunk src copies
        # (scalar HWDGE queue) so the AllReduce fires as soon as the last
        # chunk's evacuation lands.  Readbacks ride the gpsimd SWDGE
        # queue; neither waits behind bulk input loads (sync queue).
        for s in range(8):
            conv_transposed(k_in, wk_sb, kb_bc if use_kb else None,
                            kT_loc, s, s * 128)
            nc.scalar.dma_start(out=k_src[:, s], in_=kT_loc[:, s])
        ccr(k_src, k_sum)
        # fc weights deferred past the k path: the k conv (which gates
        # the collective) gets the DMA bandwidth first
        dma_w(wfc_sb, wfc_d)

        def k_peer_merge():
            # peer half into rel slots 8-15: kT[8+c] = sum[c] - kT[c].
            # Readbacks ride the sync queue (input loads are done by
            # now); the gpsimd queue stays free for collective triggers.
            for c in range(8):
                s0 = bounce.tile([128, 512], bf16, name="s0", tag="bnc")
                nc.sync.dma_start(out=s0, in_=k_sum[:, c, :])
                nc.vector.tensor_sub(kT[:, 8 + c, :], s0, kT[:, c, :])

        # ---- v conv (normal) into slotted layout (own 2 j-groups);
        # per-tile src pieces stream out as each tile's cast lands ----
        def v_tile(co, lt):
            ps = conv_ps.tile([128, 512], f32, name="convps")
            n = 0
            for t in range(3):
                for ki in range(CIN_CH):
                    nc.tensor.matmul(
                        ps,
                        lhsT=wv_sb[:, t, ki, co * 128:(co + 1) * 128],
                        rhs=v_in[:, ki, lt * 512 + t: lt * 512 + t + 512],
                        start=(n == 0), stop=(n == 11))
                    n += 1
            lo = lt * 8 * VSLOT
            dst = v_loc[:, co, lo:lo + 8 * VSLOT] \
                .rearrange("p (h e) -> p h e", e=VSLOT)[:, :, 0:D]
            src = ps.rearrange("p (h d) -> p h d", d=D)
            if use_vb:
                nc.vector.tensor_scalar_add(dst, src, vb_sb[:, co:co + 1])
            else:
                nc.vector.tensor_copy(dst, src)
            nc.scalar.dma_start(out=v_src[:, co, lo:lo + 8 * VSLOT],
                                in_=v_loc[:, co, lo:lo + 8 * VSLOT])

        def v_peer_merge():
            # peer half into rel slot blocks 2-3: sum - own (the ones
            # columns come out right automatically: 2 - 1 = 1)
            for ki in range(CIN_CH):
                for hx in range(2):
                    lo = hx * 520
                    s0 = bounce.tile([128, 520], bf16, name="s0", tag="bnc")
                    nc.sync.dma_start(out=s0, in_=v_sum[:, ki, lo:lo + 520])
                    nc.vector.tensor_sub(
                        v_slots[:, ki,
                                16 * VSLOT + lo:16 * VSLOT + lo + 520],
                        s0, v_loc[:, ki, lo:lo + 520])

        def q_slot(m, jj):
            s = jj * 4 + m
            conv_transposed(q_in, wq_sb, qb_bc if use_qb else None,
                            qT, s, s * 128)

        # ---- attention ----
        def unit_rounds(m, jj, eA, eB, fillers=(), r0=0, r1=8):
            # Heads 2m / 2m+1 share kT slots; their mm1 lhsTs sit in
            # disjoint PE row groups (partitions 0-63 / 64-127).  One
            # filler fires per round: the rounds are exp(ACT)-paced, so
            # fillers keep the in-order PE stream dense.
            qslot = jj * 4 + m
            fillers = list(fillers)
            for rnd in range(r0, r1):
                scA = scores_ps.tile([128, 2, 512], f32, name="sc")
                scB = scores_ps.tile([128, 2, 512], f32, name="sc")
                for jx in range(2):
                    c2 = CHUNK_ORDER[rnd * 2 + jx]
                    jp, ccx = c2 // 4, c2 % 4
                    slot = jp * 4 + m
                    nc.tensor.matmul(
                        scA[:, jx, :],
                        lhsT=kT[0:64, slot, ccx * 128:(ccx + 1) * 128],
                        rhs=qT[0:64, qslot, :],
                        start=True, stop=True)
                    nc.tensor.matmul(
                        scB[:, jx, :],
                        lhsT=kT[64:128, slot, ccx * 128:(ccx + 1) * 128],
                        rhs=qT[64:128, qslot, :],
                        start=True, stop=True)
                sc_scale = 1.0 / (D * WSCALE * WSCALE)
                nc.scalar.activation(
                    out=eA[:, rnd * 2:(rnd + 1) * 2, :], in_=scA,
                    func=mybir.ActivationFunctionType.Exp, scale=sc_scale)
                nc.scalar.activation(
                    out=eB[:, rnd * 2:(rnd + 1) * 2, :], in_=scB,
                    func=mybir.ActivationFunctionType.Exp, scale=sc_scale)
                if fillers:
                    fillers.pop(0)()

        def finish_head(h, jj, exp_t):
            o = o_ps.tile([128, 512], f32, name="o")
            for pos in range(KC):
                c2 = CHUNK_ORDER[pos]
                jp, ccx = c2 // 4, c2 % 4
                base = (jp * 8 + h) * VSLOT
                nc.tensor.matmul(o, lhsT=v_slots[:, ccx, base:base + 128],
                                 rhs=exp_t[:, pos, :],
                                 start=(pos == 0), stop=(pos == KC - 1))
            o_sb = o_sb_pool.tile([VSLOT, 512], bf16, name="o_sb")
            nc.vector.tensor_copy(o_sb, o[0:VSLOT, :])
            lo = 1 + jj * 512 + h * D
            for ccx in range(4):
                tp = tp_ps.tile([128, VSLOT], bf16, name="tp", tag="convps")
                nc.tensor.transpose(tp,
                                    o_sb[:, ccx * 128:(ccx + 1) * 128],
                                    ident[0:VSLOT, 0:VSLOT])
                rc = small.tile([128, 1], f32, name="rc")
                nc.vector.reciprocal(rc, tp[:, D:D + 1])
                nc.vector.tensor_scalar_mul(
                    attn_out[:, ccx, lo:lo + D], tp[:, 0:D], rc)

        def unit_alloc():
            eA = exp_pool.tile([128, KC, 512], bf16, name="exp_t")
            eB = exp_pool.tile([128, KC, 512], bf16, name="exp_t")
            return eA, eB

        def unit_finish(m, jj, eA, eB):
            finish_head(2 * m, jj, eA)
            finish_head(2 * m + 1, jj, eB)

        def pair_unit(m, jj, fillers=()):
            eA, eB = unit_alloc()
            unit_rounds(m, jj, eA, eB, fillers)
            unit_finish(m, jj, eA, eB)

        def fc_tile(co, lo, w):
            # fc output cols [lo, lo+w); reads attn_out cols lo..lo+w+1
            ps = conv_ps.tile([128, 512], f32, name="convps")
            n = 0
            for t in range(3):
                for ki in range(CIN_CH):
                    nc.tensor.matmul(
                        ps[:, 0:w],
                        lhsT=wfc_sb[:, t, ki, co * 128:(co + 1) * 128],
                        rhs=attn_out[:, ki, lo + t: lo + t + w],
                        start=(n == 0), stop=(n == 11))
                    n += 1
            fc_sb = fc_pool.tile([128, 512], f32, name="fc_sb")
            if use_fb:
                nc.vector.tensor_scalar_add(fc_sb[:, 0:w], ps[:, 0:w],
                                            fb_sb[:, co:co + 1])
            else:
                nc.vector.tensor_copy(fc_sb[:, 0:w], ps[:, 0:w])
            nc.sync.dma_start(
                out=out_d[co * 128:(co + 1) * 128, lo:lo + w],
                in_=fc_sb[:, 0:w])

        def fc_group(lo, w):
            return [(lambda co=co: fc_tile(co, lo, w)) for co in range(4)]

        # unit buffer-col ranges (attn_out col b = l' b-1):
        #   u1 (3,1): 897-1024   u2 (0,0): 1-128   u3 (1,0): 129-256
        #   u4 (2,0): 257-384    u5 (3,0): 385-512 u6 (2,1): 769-896
        #   u7 (1,1): 641-768    u8 (0,1): 513-640
        # exchange writes cols 0 / 1025 (after u1+u2, completes early).
        #
        # u1/u2's OWN-key rounds (rel chunks 0-7, rounds 0-3) need only
        # the local k conv + their q slot, so exp starts right after the
        # k conv; v tiles and the remaining q slots ride as per-round
        # fillers so the PE never idles while ACT paces the rounds.  The
        # v AllReduce triggers from inside the last v filler.
        q_slot(*UNITS[0])
        e1A, e1B = unit_alloc()
        e2A, e2B = unit_alloc()

        def vt(co, lt, trig=False):
            def f():
                v_tile(co, lt)
                if trig:
                    ccr(v_src, v_sum)
            return f

        unit_rounds(*UNITS[0], e1A, e1B, r0=0, r1=4, fillers=[
            lambda: q_slot(*UNITS[1]), vt(0, 0), vt(0, 1), vt(1, 0)])
        unit_rounds(*UNITS[1], e2A, e2B, r0=0, r1=4, fillers=[
            vt(1, 1), vt(2, 0), vt(2, 1), vt(3, 0)])
        k_peer_merge()
        # remaining conv work runs BEFORE the k-dependent peer rounds:
        # the PE chews through it while the k AllReduce + merge land, so
        # it reaches the peer matmuls with their data already in place
        # (ACT has ~85us of slack, so the exp-stream gap here is free)
        vt(3, 1, trig=True)()
        q_slot(*UNITS[2])
        q_slot(*UNITS[3])
        q_slot(*UNITS[4])
        unit_rounds(*UNITS[0], e1A, e1B, r0=4, r1=8, fillers=[
            lambda: q_slot(*UNITS[5]), lambda: q_slot(*UNITS[6])])
        unit_rounds(*UNITS[1], e2A, e2B, r0=4, r1=8, fillers=[
            lambda: q_slot(*UNITS[7])])
        v_peer_merge()
        tmp_ctx.close()
        unit_finish(*UNITS[0], e1A, e1B)
        unit_finish(*UNITS[1], e2A, e2B)

        # ---- halo exchange: send own boundary cols (buffer col 1024 =
        # l'=1023 for half 0, col 1 = l'=1024 for half 1); AllReduce-sum
        # both candidates, peer = sum - own, written under host masks.
        # mab[0] gates buffer col 0 (valid for half=1), mab[1] gates col
        # 1025 (valid for half=0). ----
        nc.gpsimd.dma_start(out=b_src[:, :, 0:1], in_=attn_out[:, :, 1:2])
        nc.gpsimd.dma_start(out=b_src[:, :, 1:2], in_=attn_out[:, :, 1024:1025])
        ccr(b_src, b_sum)
        recv = small.tile([128, CIN_CH, 2], bf16, name="recv")
        nc.sync.dma_start(out=recv, in_=b_sum)
        pb = small.tile([128, CIN_CH, 2], bf16, name="pb")
        # peer payloads: col-1024 candidate at [...,1], col-1 at [...,0]
        nc.vector.tensor_sub(pb[:, :, 0:1], recv[:, :, 0:1],
                             attn_out[:, :, 1:2])
        nc.vector.tensor_sub(pb[:, :, 1:2], recv[:, :, 1:2],
                             attn_out[:, :, 1024:1025])
        for ccx in range(4):
            # col 0 <- peer's col-1024 payload (l'=1023): index (ccx,1)
            nc.vector.tensor_scalar_mul(
                attn_out[:, ccx, 0:1], pb[:, ccx, 1:2], mab_sb[:, 0:1])
            # col 1025 <- peer's col-1 payload (l'=1024): index (ccx,0)
            nc.vector.tensor_scalar_mul(
                attn_out[:, ccx, 1025:1026], pb[:, ccx, 0:1], mab_sb[:, 1:2])

        pair_unit(*UNITS[2])                                  # u3
        pair_unit(*UNITS[3])                                  # u4
        pair_unit(*UNITS[4], fillers=fc_group(897, 127))      # u5
        pair_unit(*UNITS[5], fillers=fc_group(0, 257))        # u6
        pair_unit(*UNITS[6], fillers=fc_group(257, 254))      # u7
        pair_unit(*UNITS[7], fillers=fc_group(767, 130))      # u8
        for co in range(COUT_CH):
            fc_tile(co, 511, 256)      # reads 511..768 (u5,u8,u7): tail

    nc.compile()
    return nc


def kernel(q, k, v, wq_w, wq_b, wk_w, wk_b, wv_w, wv_b, fc_w, fc_b):
    q = np.asarray(q, np.float32)
    k = np.asarray(k, np.float32)
    v = np.asarray(v, np.float32)
    wq_w = np.asarray(wq_w, np.float32)
    wk_w = np.asarray(wk_w, np.float32)
    wv_w = np.asarray(wv_w, np.float32)
    fc_w = np.asarray(fc_w, np.float32)
    wq_b = np.asarray(wq_b, np.float32)
    wk_b = np.asarray(wk_b, np.float32)
    wv_b = np.asarray(wv_b, np.float32)
    fc_b = np.asarray(fc_b, np.float32)

    flags = (bool(wq_b.any()), bool(wk_b.any()),
             bool(wv_b.any()), bool(fc_b.any()))
    if flags not in _CACHE:
        _CACHE[flags] = _build(flags)
    nc = _CACHE[flags]
    use_qb, use_kb, use_vb, use_fb = flags

    def prep_w(w, dt=BF16, scale=1.0):
        # [Cout, Cin, 3] -> [ki, p, tap, Cout] partition-major
        wt = w.transpose(2, 1, 0).reshape(3, CIN_CH, 128, C) * scale
        return np.ascontiguousarray(wt.transpose(1, 2, 0, 3)).astype(dt)

    wq_t = prep_w(wq_w, F8, WSCALE)
    wk_t = prep_w(wk_w, F8, WSCALE)
    wv_t = prep_w(wv_w)
    wfc_t = prep_w(fc_w)

    in_maps = []
    for core in range(NCORES):
        b, half = core // 2, core % 2
        qlo = half * HALF
        qpad = np.zeros((C, L + 2), np.float32)
        qpad[:, 1:L + 1] = q[b]
        kpad = np.zeros((C, L + 2), np.float32)
        kpad[:, 1:L + 1] = k[b]
        vpad = np.zeros((C, L + 2), np.float32)
        vpad[:, 1:L + 1] = v[b]
        def prep_x(xpad, dt=BF16, w=1026):
            # [C, 1026] -> [p, ki, l] partition-major (+pad cols for fp8)
            xs = np.zeros((CIN_CH, 128, w), np.float32)
            xs[:, :, 0:1026] = xpad[:, qlo:qlo + 1026].reshape(
                CIN_CH, 128, 1026)
            return np.ascontiguousarray(xs.transpose(1, 0, 2)).astype(dt)

        m = {
            "q_in": prep_x(qpad, F8, QIN_W8),
            "k_in": prep_x(kpad, F8, QIN_W8),
            "v_in": prep_x(vpad),
            "wq": wq_t, "wk": wk_t, "wv": wv_t, "wfc": wfc_t,
            # mab[0] gates buffer col 0 (l'=1023, valid for half=1);
            # mab[1] gates col 1025 (l'=1024, valid for half=0)
            "mab": np.array([[float(half == 1), float(half == 0)]],
                            np.float32),
        }
        if use_qb:
            m["qb"] = wq_b.reshape(1, C) * WSCALE
        if use_kb:
            m["kb"] = wk_b.reshape(1, C) * WSCALE
        if use_vb:
            m["vb"] = np.ascontiguousarray(wv_b.reshape(CIN_CH, 128).T)
        if use_fb:
            m["fb"] = np.ascontiguousarray(fc_b.reshape(CIN_CH, 128).T)
        in_maps.append(m)

    global _LAST_IN_MAPS, _LAST_RESULTS
    _LAST_IN_MAPS = in_maps
    from concourse.bass_utils import run_bass_kernel_spmd
    res = run_bass_kernel_spmd(nc, in_maps, list(range(NCORES))).results
    _LAST_RESULTS = res

    out = np.empty((B, C, L), np.float32)
    for core in range(NCORES):
        b, half = core // 2, core % 2
        out[b][:, half * HALF:(half + 1) * HALF] = res[core]["out"]
    return out
